# revision 20
# baseline (speedup 1.0000x reference)
"""Self-contained Trainium2 (Bass/Tile) kernel: single-head causal attention.

Problem: embeddings [4,4096,1024] f32; Wq/Wk/Wv [1024,1024] f32 (torch Linear
layout [out,in]).  out = softmax(causal(QK^T)/sqrt(D)) @ V, computed per batch.

Distribution: 8 NeuronCores, one SPMD program.  Core c handles batch c//2 and
16 query chunks of 128 rows.  Causal load-balance with a uniform program:
slot j (j=0..15) processes nkt_j = 2j+2 K-tiles (128 rows each); the core
with parity p takes q-chunk 2j+p (rows 128*(2j+p)..+128).  Parity 0 needs
2j+1 tiles (the extra one is masked to zero), parity 1 needs exactly 2j+2.
Per-core differences (batch data, q-row selection, causal masks) enter via
input data only, so all 8 cores run the same NEFF.

scores = Q K^T = emb_q (Wq^T Wk) emb_k^T, so K is never materialized:
M = Wq^T Wk once, qt = (emb_q M)^T, scores tile = embT_tile^T @ qt_tile.

V is never materialized either: AV = W (emb Wv^T) is re-associated as
(W emb) Wv^T.  Per slot, U^T[d,q] = sum_k emb[k,d] w[k,q] accumulates in
PSUM over the slot's K-tiles (8 stationary = emb_nat d-slices, moving = w),
then out = (U Wv^T)/l via a 16-matmul second stage.  This removes the
duplicated 110us V projection (each pair computed the same V) and replaces
it with a 55us per-slot stage, saving ~55us of PE time per core.

Host-side prep (layout only, no model math): transpose/reshape + bf16-cast
of inputs, q-row gather, mask table construction.  All projections, scores,
softmax and AV run on device (bf16 matmuls, f32 accumulation/softmax).

DMA: few large multi-dim transfers spread over 4 engine queues (sync:embT,
vector:wq+embq, scalar:wk+wv+masks, gpsimd:emb_nat), instead of ~100 small
triggers on 2 queues (the old version lost ~30us to trigger serialization).
emb_nat SBUF space is tag-aliased onto wq/wk (dead after M) and m/embq
(dead after qt).  A short dummy-matmul warmup at t~0 flips the PE HAM
throttle to 2.4GHz before the real matmuls start.
"""

import math
import os
import sys
import types

import numpy as np
import ml_dtypes

B, S, D = 4, 4096, 1024
NCORES = 8
NSLOT = 16
CHUNK = 128          # q rows per slot
NKT = [2 * j + 2 for j in range(NSLOT)]   # K-tiles (128 rows) per slot
INV_SQRT_D = 1.0 / math.sqrt(D)
BF16 = ml_dtypes.bfloat16


# ---------------------------------------------------------------------------
# Environment patches (compiler workarounds + profiling hook shim)
# ---------------------------------------------------------------------------

def _install_patches():
    import json as _json
    import concourse.bass as bass

    if not getattr(bass.Bass, "_mw_patched", False):
        _orig_to_json = bass.Bass.to_json_bytes

        def to_json_bytes(self):
            # This walrus build rejects any instruction carrying more than one
            # sync wait ("Too many sync wait commands").  Split extra waits
            # onto single-wait NoOps inserted just before the instruction (the
            # engine executes them in order, so semantics are unchanged).
            raw = _orig_to_json(self)
            m = _json.loads(raw)
            ctr = 0
            changed = False
            for fn in m.get("functions", []):
                for bb in fn.get("blocks", []):
                    out = []
                    for inst in bb.get("instructions", []):
                        si = inst.get("sync_info")
                        if si:
                            waits = si.get("on_wait") or []
                            if len(waits) > 1:
                                changed = True
                                for w in waits[:-1]:
                                    ctr += 1
                                    out.append({
                                        "debug": inst.get("debug", 0),
                                        "engine": inst["engine"],
                                        "ins": [],
                                        "outs": [],
                                        "name": f"I-mw{ctr}",
                                        "opcode": "NoOp",
                                        "text_hint": "mwsplit",
                                        "sync_info": {"on_wait": [w],
                                                      "on_update": []},
                                    })
                                si["on_wait"] = [waits[-1]]
                        out.append(inst)
                    bb["instructions"] = out
            if not changed:
                return raw
            return _json.dumps(m).encode()

        bass.Bass.to_json_bytes = to_json_bytes
        bass.Bass._mw_patched = True

    # Don't upload NEFF/trace artifacts anywhere; keep them local.
    import concourse.bass_utils as bu
    bu.upload_artifacts = lambda tmpdir: tmpdir


def _install_ntff_hook() -> bool:
    """Register the axon NTFF profiling hook (missing module in this image)."""
    try:
        import antenv.axon_hooks  # noqa: F401
        return True
    except ImportError:
        pass
    try:
        mod = types.ModuleType("antenv.axon_hooks")
        state = {"hook": None}
        mod.set_axon_ntff_profile_hook = lambda h: state.__setitem__("hook", h)
        mod.get_axon_ntff_profile_hook = lambda: state["hook"]
        sys.modules["antenv.axon_hooks"] = mod
        import antenv
        antenv.axon_hooks = mod
        from trn_agent_boot.trn_boot import _ntff_profile_via_ctypes
        mod.set_axon_ntff_profile_hook(
            _ntff_profile_via_ctypes("/opt/axon/libaxon_pjrt.so"))
        return True
    except Exception:
        return False


# ---------------------------------------------------------------------------
# Graph
# ---------------------------------------------------------------------------

def _build_graph():
    import concourse.bass as bass
    import concourse.mybir as mybir
    import concourse.tile as tile

    f32 = mybir.dt.float32
    bf16 = mybir.dt.bfloat16
    Exp = mybir.ActivationFunctionType.Exp
    Copy = mybir.ActivationFunctionType.Copy

    nc = bass.Bass("TRN2", debug=False, num_devices=NCORES)

    embT_in = nc.dram_tensor("embTq", [4, 128, 8, 1024], bf16,
                             kind="ExternalInput")
    ennat_in = nc.dram_tensor("ennat", [32, 128, 1024], bf16,
                              kind="ExternalInput")
    embq_in = nc.dram_tensor("embqh", [4, 128, 4, 1024], bf16,
                             kind="ExternalInput")
    wq_in = nc.dram_tensor("wqh", [2, 128, 4, 1024], bf16,
                           kind="ExternalInput")
    wk_in = nc.dram_tensor("wkh", [2, 128, 4, 1024], bf16,
                           kind="ExternalInput")
    wv_in = nc.dram_tensor("wvh", [2, 128, 4, 1024], bf16,
                           kind="ExternalInput")
    masks_in = nc.dram_tensor("masks", [2, 128, CHUNK], bf16,
                              kind="ExternalInput")
    out_d = nc.dram_tensor("out", [NSLOT * CHUNK, D], bf16,
                           kind="ExternalOutput")
    debug = bool(int(os.environ.get("BASS_DEBUG_DUMP", "0")))
    if debug:
        dbgr_d = nc.dram_tensor("dbgr", [NSLOT, 128, 1], f32,
                                kind="ExternalOutput")
        dbgu_d = nc.dram_tensor("dbgu", [NSLOT, 128, 128], bf16,
                                kind="ExternalOutput")
        dbgw_d = nc.dram_tensor("dbgw", [NSLOT, 128, CHUNK], bf16,
                                kind="ExternalOutput")
        dbgq_d = nc.dram_tensor("dbgq", [128, 128], bf16,
                                kind="ExternalOutput")

    with tile.TileContext(nc) as tc:
        with (
            tc.tile_pool(name="big", bufs=1) as big,        # quads + residents
            tc.tile_pool(name="wts", bufs=3) as wts,        # exp weights
            tc.tile_pool(name="outs", bufs=2) as outs,      # output stage
            tc.tile_pool(name="smalls", bufs=2) as smalls,
            tc.tile_pool(name="pso", bufs=1, space="PSUM") as pso,    # o0/o1
            tc.tile_pool(name="put", bufs=1, space="PSUM") as put,    # utg0/1
            tc.tile_pool(name="ps", bufs=2, space="PSUM") as ps_pool,  # s
            tc.tile_pool(name="pl", bufs=1, space="PSUM") as pl_pool,  # l
        ):
            # ---- constants + PE warmup ----------------------------------
            ones = smalls.tile([128, 1], bf16, name="ones", tag="ones")
            nc.gpsimd.memset(ones[:], 1.0)
            wu = smalls.tile([128, 128], bf16, name="wu", tag="wu")
            nc.gpsimd.memset(wu[:], 0.25)
            # ~80 matmuls of N=128 keep the PE busy from ~t=0 so the HAM
            # clock gate flips to 2.4GHz before the first real matmul, and
            # the wq/wk DMA ramp (~12us) hides behind them.
            wu_ps = ps_pool.tile([128, 128], f32, name="wups", tag="s")
            for i in range(160):
                nc.tensor.matmul(wu_ps[:], wu[:], wu[:],
                                 start=True, stop=True)

            # ---- input DMAs (few, large, spread over 3 queues) ----------
            # M is gated only on wq+wk (4MB split: wq on gpsimd, wk on
            # scalar), so it starts ~12us in, under the warmup.  wv+embT
            # ride the sync queue (not needed until attention at ~115us).
            # Critical path first: M is gated on wq+wk (4MB, one 1MB quad
            # per queue + wk1 behind wq1 on sync).  Everything not needed
            # until qt/attention (eq, wv, embT) is chained BEHIND the
            # critical quads on each queue so it doesn't steal HBM
            # bandwidth during the first ~20us.
            wqQ = [big.tile([128, 4, 1024], bf16, name=f"wq{i}",
                            tag=f"wq{i}") for i in range(2)]
            wkQ = [big.tile([128, 4, 1024], bf16, name=f"wk{i}",
                            tag=f"wk{i}") for i in range(2)]
            wvQ = [big.tile([128, 4, 1024], bf16, name=f"wv{i}",
                            tag=f"wv{i}") for i in range(2)]
            qeng = [nc.gpsimd, nc.scalar, nc.sync]
            for hi, (dst, src) in enumerate(
                    [(wqQ[0], wq_in[0]), (wqQ[1], wq_in[1]),
                     (wkQ[0], wk_in[0]), (wkQ[1], wk_in[1])]):
                for h in range(2):
                    qeng[(2 * hi + h) % 3].dma_start(
                        dst[:, 2 * h:2 * h + 2, :], src[:, 2 * h:2 * h + 2, :])

            # embq blocks 0/1 early (qt consumes them right after M);
            # blocks 2/3 stream behind them, reusing the same two buffers.
            eqs = [big.tile([128, 4, 1024], bf16, name=f"eq{qb}",
                            tag=f"eq{qb % 2}") for qb in range(2)]
            for qb in range(2):
                nc.gpsimd.dma_start(eqs[qb][:, :, :], embq_in[qb, :, :, :])

            mask_sb = []
            for t in range(2):
                mt = smalls.tile([128, CHUNK], bf16, name=f"mk{t}",
                                 tag=f"mk{t}")
                nc.scalar.dma_start(mt[:], masks_in[t, :, :])
                mask_sb.append(mt)
            for i in range(2):
                nc.scalar.dma_start(wvQ[i][:, :, :], wv_in[i, :, :, :])

            embt = big.tile([128, 8, 4096], bf16, name="embt", tag="embt")
            for q in range(4):
                nc.scalar.dma_start(embt[:, :, q * 1024:(q + 1) * 1024],
                                    embT_in[q, :, :, :])

            def wq_sl(ec, a0, n):      # [128e, n] slice of Wq rows ec-chunk
                return wqQ[ec // 4][:, ec % 4, a0:a0 + n]

            def wk_sl(ec, b0, n):
                return wkQ[ec // 4][:, ec % 4, b0:b0 + n]

            def wv_sl(dc, e0, n):
                return wvQ[dc // 4][:, dc % 4, e0:e0 + n]

            def embt_sl(dc, k0, n):    # [128d, n] slice of emb^T
                return embt[:, dc, k0:k0 + n]

            # ---------------- M = Wq^T @ Wk  [d_a, d_b] ------------------
            # Computed fully on every core: the 8-rank AllGather floor in
            # this environment is ~90us, which cannot hide anywhere between
            # M and qt (its only consumer), so splitting M is a net loss.
            mQ = [big.tile([128, 4, 1024], bf16, name=f"m{i}", tag=f"m{i}")
                  for i in range(2)]
            for ac in range(8):
                for bb in range(2):
                    psum = pso.tile([128, 512], f32, name=f"pm{ac}_{bb}",
                                    tag=f"o{bb}")
                    for ec in range(8):
                        nc.tensor.matmul(
                            psum[:], wq_sl(ec, ac * 128, 128),
                            wk_sl(ec, bb * 512, 512),
                            start=(ec == 0), stop=(ec == 7))
                    nc.scalar.copy(
                        mQ[ac // 4][:, ac % 4, bb * 512:(bb + 1) * 512],
                        psum[:])

            def m_sl(ac, b0, n):
                return mQ[ac // 4][:, ac % 4, b0:b0 + n]

            # emb_nat K-tiles 0..15 overwrite wq/wk (dead after M).
            # en[k] holds K-tiles 4k..4k+3 as [128part, 4, 1024d].
            # (en[4..7] alias m/eq and are created after the qt loop so the
            # pool's per-tag alias order matches program order.)
            en_tags = ["wq0", "wq1", "wk0", "wk1", "m0", "m1", "eq0", "eq1"]
            en = [big.tile([128, 4, 1024], bf16, name=f"en{k}",
                           tag=en_tags[k]) for k in range(4)]
            for k in range(4):
                for j in range(4):
                    nc.gpsimd.dma_start(en[k][:, j, :],
                                        ennat_in[4 * k + j, :, :])

            def en_sl(kt, d0, n):      # [128k, n] slice of natural emb
                return en[kt // 4][:, kt % 4, d0:d0 + n]

            # ------------- qt = (emb_q M)^T, kept in SBUF ----------------
            qt_sb = [big.tile([128, 1024], bf16, name=f"qt{i}", tag=f"qt{i}")
                     for i in range(16)]
            for qb in range(4):
                if qb >= 2:
                    eqs.append(big.tile([128, 4, 1024], bf16, name=f"eq{qb}",
                                        tag=f"eq{qb % 2}"))
                    nc.gpsimd.dma_start(eqs[qb][:, :, :],
                                        embq_in[qb, :, :, :])
                eq = eqs[qb]
                half, off = qb // 2, (qb % 2) * 512
                for bc in range(8):
                    psum = pso.tile([128, 512], f32, name=f"pq{qb}_{bc}",
                                    tag=f"o{bc % 2}")
                    for ac in range(8):
                        nc.tensor.matmul(
                            psum[:], m_sl(ac, bc * 128, 128),
                            eq[:, ac // 2, (ac % 2) * 512:(ac % 2) * 512 + 512],
                            start=(ac == 0), stop=(ac == 7))
                    nc.scalar.copy(qt_sb[bc * 2 + half][:, off:off + 512],
                                   psum[:])

            if debug:
                nc.gpsimd.dma_start(dbgq_d[:, :], qt_sb[0][:, 0:128])

            # emb_nat K-tiles 16..31 overwrite m/eq (dead after qt).
            en += [big.tile([128, 4, 1024], bf16, name=f"en{k}",
                            tag=en_tags[k]) for k in range(4, 8)]
            for k in range(4, 8):
                for j in range(4):
                    nc.gpsimd.dma_start(en[k][:, j, :],
                                        ennat_in[4 * k + j, :, :])

            # ---------------- attention ----------------
            # Emission order alternates long and short slots so a short
            # slot's boundary latency hides under the following long slot's
            # score stream.
            order = []
            for i in range(NSLOT // 2):
                order += [NSLOT - 1 - i, i]

            # U^T/l accumulation trails the score stream by two tiles (the
            # pend queue), and each slot's second stage (U Wv^T) is emitted
            # two further drains later so its PSUM->SBUF copies (vector/
            # gpsimd) complete under the next slot's scores.  PSUM is bank-
            # granular: utg 2 banks + o0/o1 2 + s x2 2 + l x2 2 = 8, so
            # the U^T group is single-buffered (per-slice WAR deps let
            # the next slot's accumulation start as each copy completes).
            # The two l accumulators must be separate tiles: column-sliced
            # sharing of one PSUM tile corrupts the neighboring column.
            pend = []
            stage2q = []

            def emit_stage2(j, utsb, r_sb):
                row = j * CHUNK
                for eb in range(2):
                    op = pso.tile([128, 512], f32, name=f"po{j}_{eb}",
                                  tag=f"o{eb}")
                    for dc in range(8):
                        nc.tensor.matmul(
                            op[:], utsb[dc][:],
                            wv_sl(dc, eb * 512, 512),
                            start=(dc == 0), stop=(dc == 7))
                    o_sb = outs.tile([128, 512], bf16, name=f"o{j}_{eb}",
                                     tag=f"os{eb}")
                    if eb == 0:
                        nc.scalar.activation(o_sb[:], op[:], Copy, bias=0.0,
                                             scale=r_sb[:])
                    else:
                        nc.vector.tensor_scalar_mul(o_sb[:], op[:], r_sb[:])
                    nc.sync.dma_start(
                        out_d[row:row + CHUNK, eb * 512:(eb + 1) * 512],
                        o_sb[:])

            def drain_one():
                wt, kt, j, grp, ut, l_ps, nkt = pend.pop(0)
                first, last = kt == 0, kt == nkt - 1
                for ds in range(8):
                    # start=True clears has_written for the WHOLE PSUM bank,
                    # not just this region, so only the first region of each
                    # bank (ds 0 and 4) may carry it.  The bank-wide clear
                    # leaves the other regions' bits unset, so their first
                    # write overwrites (not accumulates) as needed.
                    nc.tensor.matmul(
                        ut[:, ds * 128:(ds + 1) * 128],
                        en_sl(kt, ds * 128, 128), wt[:],
                        start=(first and ds % 4 == 0), stop=last,
                        skip_group_check=True)
                # l rowsum shares wt as the stationary operand; issued last
                # so its weight load prefetches under the U^T streams.
                nc.tensor.matmul(l_ps[:], wt[:], ones[:],
                                 start=first, stop=last)
                if last:
                    r_sb = smalls.tile([128, 1], f32, name=f"r{j}",
                                       tag=f"r{grp}")
                    nc.vector.reciprocal(r_sb[:], l_ps[:])
                    utsb = [big.tile([128, 128], bf16, name=f"uts{j}_{ds}",
                                     tag=f"uts{ds}") for ds in range(8)]
                    for ds in range(8):
                        if ds % 2 == 0:
                            nc.vector.tensor_copy(
                                utsb[ds][:], ut[:, ds * 128:(ds + 1) * 128])
                        else:
                            nc.scalar.copy(
                                utsb[ds][:], ut[:, ds * 128:(ds + 1) * 128])
                    if debug:
                        nc.sync.dma_start(dbgr_d[j, :, :], r_sb[:])
                        nc.gpsimd.dma_start(dbgu_d[j, :, :], utsb[0][:])
                    stage2q.append([2, (j, utsb, r_sb)])
                for e2 in stage2q:
                    e2[0] -= 1
                while stage2q and stage2q[0][0] <= 0:
                    _, (j2, u2, r2) = stage2q.pop(0)
                    emit_stage2(j2, u2, r2)

            for e, j in enumerate(order):
                nkt = NKT[j]
                h, c0 = j // 8, (j % 8) * CHUNK
                qt_tiles = [qt_sb[ec * 2 + h][:, c0:c0 + CHUNK]
                            for ec in range(8)]
                grp = e % 2
                ut = put.tile([128, 1024], f32, name=f"ut{j}", tag="utg")
                l_ps = pl_pool.tile([128, 1], f32, name=f"l{j}",
                                    tag=f"l{grp}")

                for kt in range(nkt):
                    s_ps = ps_pool.tile([128, CHUNK], f32, name=f"s{j}_{kt}",
                                        tag="s")
                    for dc in range(8):
                        nc.tensor.matmul(
                            s_ps[:], embt_sl(dc, kt * 128, 128),
                            qt_tiles[dc], start=(dc == 0), stop=(dc == 7))

                    wt = wts.tile([128, CHUNK], bf16, name=f"w{j}_{kt}",
                                  tag="wts")
                    nc.scalar.activation(wt[:], s_ps[:], Exp, bias=0.0,
                                         scale=INV_SQRT_D)
                    if kt >= nkt - 2:
                        nc.vector.tensor_mul(wt[:], wt[:],
                                             mask_sb[kt - (nkt - 2)][:])
                    if debug and kt == 0:
                        nc.gpsimd.dma_start(dbgw_d[j, :, :], wt[:])
                    pend.append((wt, kt, j, grp, ut, l_ps, nkt))
                    if len(pend) > 2:
                        drain_one()
            while pend:
                drain_one()
            while stage2q:
                _, (j2, u2, r2) = stage2q.pop(0)
                emit_stage2(j2, u2, r2)

    return nc


_CACHED = {}


def _get_graph():
    if "nc" not in _CACHED:
        _install_patches()
        _CACHED["nc"] = _build_graph()
    return _CACHED["nc"]


# ---------------------------------------------------------------------------
# Host-side staging (layout only)
# ---------------------------------------------------------------------------

def _chunks(parity):
    return [2 * j + parity for j in range(NSLOT)]


def _masks(parity):
    # mask tile t (t=0,1) applies to K-tile (2j+t) of slot j:
    # keep iff 128*parity + x >= 128*t + r  (r = k row in partition dim,
    # x = q col in free dim).
    m = np.zeros((2, 128, CHUNK), dtype=np.float32)
    r = np.arange(128)[:, None]
    x = np.arange(CHUNK)[None, :]
    for t in range(2):
        m[t] = ((128 * parity + x) >= (128 * t + r)).astype(np.float32)
    return m.astype(BF16)


def _quads(w):
    # [1024, 1024] row-chunked to [2, 128, 4, 1024] quad layout
    return np.ascontiguousarray(
        w.reshape(2, 4, 128, 1024).transpose(0, 2, 1, 3))


def kernel(embeddings, Wq, Wk, Wv):
    embeddings = np.asarray(embeddings, dtype=np.float32)
    Wq = np.asarray(Wq, dtype=np.float32)
    Wk = np.asarray(Wk, dtype=np.float32)
    Wv = np.asarray(Wv, dtype=np.float32)

    nc = _get_graph()
    from concourse.bass_utils import run_bass_kernel_spmd

    wqh = _quads(Wq).astype(BF16)
    wkh = _quads(Wk).astype(BF16)
    wvh = _quads(np.ascontiguousarray(Wv.T)).astype(BF16)
    masks_by_par = [_masks(0), _masks(1)]

    in_maps = []
    for c in range(NCORES):
        b, par = divmod(c, 2)
        emb_b = embeddings[b]
        embT = emb_b.T                                    # [1024, 4096]
        embTq = np.ascontiguousarray(
            embT.reshape(8, 128, 4, 1024).transpose(2, 1, 0, 3)).astype(BF16)
        ennat = np.ascontiguousarray(
            emb_b.reshape(32, 128, 1024)).astype(BF16)
        rows = np.concatenate(
            [np.arange(g * CHUNK, (g + 1) * CHUNK) for g in _chunks(par)])
        eqT = np.ascontiguousarray(emb_b[rows].T)         # [1024, 2048]
        embqh = np.ascontiguousarray(
            eqT.reshape(4, 2, 128, 4, 512).transpose(3, 2, 0, 1, 4)
            .reshape(4, 128, 4, 1024)).astype(BF16)
        in_maps.append({
            "embTq": embTq,
            "ennat": ennat,
            "embqh": embqh,
            "wqh": wqh,
            "wkh": wkh,
            "wvh": wvh,
            "masks": masks_by_par[par],
        })

    trace = bool(int(os.environ.get("BASS_KERNEL_TRACE", "0")))
    kwargs = {}
    if trace:
        kwargs["trace"] = _install_ntff_hook()

    res = run_bass_kernel_spmd(nc, in_maps, core_ids=list(range(NCORES)),
                               **kwargs)
    _CACHED["last_result"] = res

    out = np.empty((B, S, D), dtype=np.float32)
    for c in range(NCORES):
        b, par = divmod(c, 2)
        core_out = res.results[c]["out"].astype(np.float32)
        for j, g in enumerate(_chunks(par)):
            out[b, g * CHUNK:(g + 1) * CHUNK] = \
                core_out[j * CHUNK:(j + 1) * CHUNK]
    return out


# revision 21
# speedup vs baseline: 1.0359x; 1.0359x over previous
"""Self-contained Trainium2 (Bass/Tile) kernel: single-head causal attention.

Problem: embeddings [4,4096,1024] f32; Wq/Wk/Wv [1024,1024] f32 (torch Linear
layout [out,in]).  out = softmax(causal(QK^T)/sqrt(D)) @ V, computed per batch.

Distribution: 8 NeuronCores, one SPMD program.  Core c handles batch c//2 and
16 query chunks of 128 rows.  Causal load-balance with a uniform program:
slot j (j=0..15) processes nkt_j = 2j+2 K-tiles (128 rows each); the core
with parity p takes q-chunk 2j+p (rows 128*(2j+p)..+128).  Parity 0 needs
2j+1 tiles (the extra one is masked to zero), parity 1 needs exactly 2j+2.
Per-core differences (batch data, q-row selection, causal masks) enter via
input data only, so all 8 cores run the same NEFF.

scores = Q K^T = emb_q (Wq^T Wk) emb_k^T, so K is never materialized:
M = Wq^T Wk once, qt = (emb_q M)^T, scores tile = embT_tile^T @ qt_tile.

V is never materialized either: AV = W (emb Wv^T) is re-associated as
(W emb) Wv^T.  Per slot, U^T[d,q] = sum_k emb[k,d] w[k,q] accumulates in
PSUM over the slot's K-tiles (8 stationary = emb_nat d-slices, moving = w),
then out = (U Wv^T)/l via a 16-matmul second stage.  This removes the
duplicated 110us V projection (each pair computed the same V) and replaces
it with a 55us per-slot stage, saving ~55us of PE time per core.

Host-side prep (layout only, no model math): transpose/reshape + bf16-cast
of inputs, q-row gather, mask table construction.  All projections, scores,
softmax and AV run on device (bf16 matmuls, f32 accumulation/softmax).

DMA: few large multi-dim transfers spread over 4 engine queues (sync:embT,
vector:wq+embq, scalar:wk+wv+masks, gpsimd:emb_nat), instead of ~100 small
triggers on 2 queues (the old version lost ~30us to trigger serialization).
emb_nat SBUF space is tag-aliased onto wq/wk (dead after M) and m/embq
(dead after qt).  A short dummy-matmul warmup at t~0 flips the PE HAM
throttle to 2.4GHz before the real matmuls start.
"""

import math
import os
import sys
import types

import numpy as np
import ml_dtypes

B, S, D = 4, 4096, 1024
NCORES = 8
NSLOT = 16
CHUNK = 128          # q rows per slot
NKT = [2 * j + 2 for j in range(NSLOT)]   # K-tiles (128 rows) per slot
INV_SQRT_D = 1.0 / math.sqrt(D)
BF16 = ml_dtypes.bfloat16


# ---------------------------------------------------------------------------
# Environment patches (compiler workarounds + profiling hook shim)
# ---------------------------------------------------------------------------

def _install_patches():
    import json as _json
    import concourse.bass as bass

    if not getattr(bass.Bass, "_mw_patched", False):
        _orig_to_json = bass.Bass.to_json_bytes

        def to_json_bytes(self):
            # This walrus build rejects any instruction carrying more than one
            # sync wait ("Too many sync wait commands").  Split extra waits
            # onto single-wait NoOps inserted just before the instruction (the
            # engine executes them in order, so semantics are unchanged).
            raw = _orig_to_json(self)
            m = _json.loads(raw)
            ctr = 0
            changed = False
            for fn in m.get("functions", []):
                for bb in fn.get("blocks", []):
                    out = []
                    for inst in bb.get("instructions", []):
                        si = inst.get("sync_info")
                        if si:
                            waits = si.get("on_wait") or []
                            if len(waits) > 1:
                                changed = True
                                for w in waits[:-1]:
                                    ctr += 1
                                    out.append({
                                        "debug": inst.get("debug", 0),
                                        "engine": inst["engine"],
                                        "ins": [],
                                        "outs": [],
                                        "name": f"I-mw{ctr}",
                                        "opcode": "NoOp",
                                        "text_hint": "mwsplit",
                                        "sync_info": {"on_wait": [w],
                                                      "on_update": []},
                                    })
                                si["on_wait"] = [waits[-1]]
                        out.append(inst)
                    bb["instructions"] = out
            if not changed:
                return raw
            return _json.dumps(m).encode()

        bass.Bass.to_json_bytes = to_json_bytes
        bass.Bass._mw_patched = True

    # Don't upload NEFF/trace artifacts anywhere; keep them local.
    import concourse.bass_utils as bu
    bu.upload_artifacts = lambda tmpdir: tmpdir


def _install_ntff_hook() -> bool:
    """Register the axon NTFF profiling hook (missing module in this image)."""
    try:
        import antenv.axon_hooks  # noqa: F401
        return True
    except ImportError:
        pass
    try:
        mod = types.ModuleType("antenv.axon_hooks")
        state = {"hook": None}
        mod.set_axon_ntff_profile_hook = lambda h: state.__setitem__("hook", h)
        mod.get_axon_ntff_profile_hook = lambda: state["hook"]
        sys.modules["antenv.axon_hooks"] = mod
        import antenv
        antenv.axon_hooks = mod
        from trn_agent_boot.trn_boot import _ntff_profile_via_ctypes
        mod.set_axon_ntff_profile_hook(
            _ntff_profile_via_ctypes("/opt/axon/libaxon_pjrt.so"))
        return True
    except Exception:
        return False


# ---------------------------------------------------------------------------
# Graph
# ---------------------------------------------------------------------------

def _build_graph():
    import concourse.bass as bass
    import concourse.mybir as mybir
    import concourse.tile as tile

    f32 = mybir.dt.float32
    bf16 = mybir.dt.bfloat16
    Exp = mybir.ActivationFunctionType.Exp
    Copy = mybir.ActivationFunctionType.Copy

    nc = bass.Bass("TRN2", debug=False, num_devices=NCORES)

    embT_in = nc.dram_tensor("embTq", [4, 128, 8, 1024], bf16,
                             kind="ExternalInput")
    ennat_in = nc.dram_tensor("ennat", [32, 128, 1024], bf16,
                              kind="ExternalInput")
    embq_in = nc.dram_tensor("embqh", [4, 128, 4, 1024], bf16,
                             kind="ExternalInput")
    wq_in = nc.dram_tensor("wqh", [2, 128, 4, 1024], bf16,
                           kind="ExternalInput")
    wk_in = nc.dram_tensor("wkh", [2, 128, 4, 1024], bf16,
                           kind="ExternalInput")
    wv_in = nc.dram_tensor("wvh", [2, 128, 4, 1024], bf16,
                           kind="ExternalInput")
    masks_in = nc.dram_tensor("masks", [2, 128, CHUNK], bf16,
                              kind="ExternalInput")
    out_d = nc.dram_tensor("out", [NSLOT * CHUNK, D], bf16,
                           kind="ExternalOutput")
    debug = bool(int(os.environ.get("BASS_DEBUG_DUMP", "0")))
    if debug:
        dbgr_d = nc.dram_tensor("dbgr", [NSLOT, 128, 1], f32,
                                kind="ExternalOutput")
        dbgu_d = nc.dram_tensor("dbgu", [NSLOT, 128, 128], bf16,
                                kind="ExternalOutput")
        dbgw_d = nc.dram_tensor("dbgw", [NSLOT, 128, CHUNK], bf16,
                                kind="ExternalOutput")
        dbgq_d = nc.dram_tensor("dbgq", [128, 128], bf16,
                                kind="ExternalOutput")

    with tile.TileContext(nc) as tc:
        with (
            tc.tile_pool(name="big", bufs=1) as big,        # quads + residents
            tc.tile_pool(name="wts", bufs=3) as wts,        # exp weights
            tc.tile_pool(name="outs", bufs=2) as outs,      # output stage
            tc.tile_pool(name="smalls", bufs=2) as smalls,
            tc.tile_pool(name="pso", bufs=1, space="PSUM") as pso,    # o0/o1
            tc.tile_pool(name="put", bufs=1, space="PSUM") as put,    # utg0/1
            tc.tile_pool(name="ps", bufs=2, space="PSUM") as ps_pool,  # s
            tc.tile_pool(name="pl", bufs=1, space="PSUM") as pl_pool,  # l
        ):
            # ---- constants + PE warmup ----------------------------------
            ones = smalls.tile([128, 1], bf16, name="ones", tag="ones")
            nc.gpsimd.memset(ones[:], 1.0)
            wu = smalls.tile([128, 128], bf16, name="wu", tag="wu")
            nc.gpsimd.memset(wu[:], 0.25)
            # ~80 matmuls of N=128 keep the PE busy from ~t=0 so the HAM
            # clock gate flips to 2.4GHz before the first real matmul, and
            # the wq/wk DMA ramp (~12us) hides behind them.
            wu_ps = ps_pool.tile([128, 128], f32, name="wups", tag="s")
            for i in range(160):
                nc.tensor.matmul(wu_ps[:], wu[:], wu[:],
                                 start=True, stop=True)

            # ---- input DMAs (few, large, spread over 3 queues) ----------
            # M is gated only on wq+wk (4MB split: wq on gpsimd, wk on
            # scalar), so it starts ~12us in, under the warmup.  wv+embT
            # ride the sync queue (not needed until attention at ~115us).
            # Critical path first: M is gated on wq+wk (4MB, one 1MB quad
            # per queue + wk1 behind wq1 on sync).  Everything not needed
            # until qt/attention (eq, wv, embT) is chained BEHIND the
            # critical quads on each queue so it doesn't steal HBM
            # bandwidth during the first ~20us.
            wqQ = [big.tile([128, 4, 1024], bf16, name=f"wq{i}",
                            tag=f"wq{i}") for i in range(2)]
            wkQ = [big.tile([128, 4, 1024], bf16, name=f"wk{i}",
                            tag=f"wk{i}") for i in range(2)]
            wvQ = [big.tile([128, 4, 1024], bf16, name=f"wv{i}",
                            tag=f"wv{i}") for i in range(2)]
            qeng = [nc.gpsimd, nc.scalar, nc.sync]
            for hi, (dst, src) in enumerate(
                    [(wqQ[0], wq_in[0]), (wqQ[1], wq_in[1]),
                     (wkQ[0], wk_in[0]), (wkQ[1], wk_in[1])]):
                for h in range(2):
                    qeng[(2 * hi + h) % 3].dma_start(
                        dst[:, 2 * h:2 * h + 2, :], src[:, 2 * h:2 * h + 2, :])

            # embq blocks 0/1 early (qt consumes them right after M);
            # blocks 2/3 stream behind them, reusing the same two buffers.
            eqs = [big.tile([128, 4, 1024], bf16, name=f"eq{qb}",
                            tag=f"eq{qb % 2}") for qb in range(2)]
            for qb in range(2):
                nc.gpsimd.dma_start(eqs[qb][:, :, :], embq_in[qb, :, :, :])

            mask_sb = []
            for t in range(2):
                mt = smalls.tile([128, CHUNK], bf16, name=f"mk{t}",
                                 tag=f"mk{t}")
                nc.scalar.dma_start(mt[:], masks_in[t, :, :])
                mask_sb.append(mt)
            for i in range(2):
                nc.scalar.dma_start(wvQ[i][:, :, :], wv_in[i, :, :, :])

            embt = big.tile([128, 8, 4096], bf16, name="embt", tag="embt")
            for q in range(4):
                nc.sync.dma_start(embt[:, :, q * 1024:(q + 1) * 1024],
                                  embT_in[q, :, :, :])

            def wq_sl(ec, a0, n):      # [128e, n] slice of Wq rows ec-chunk
                return wqQ[ec // 4][:, ec % 4, a0:a0 + n]

            def wk_sl(ec, b0, n):
                return wkQ[ec // 4][:, ec % 4, b0:b0 + n]

            def wv_sl(dc, e0, n):
                return wvQ[dc // 4][:, dc % 4, e0:e0 + n]

            def embt_sl(dc, k0, n):    # [128d, n] slice of emb^T
                return embt[:, dc, k0:k0 + n]

            # ---------------- M = Wq^T @ Wk  [d_a, d_b] ------------------
            # Computed fully on every core: the 8-rank AllGather floor in
            # this environment is ~90us, which cannot hide anywhere between
            # M and qt (its only consumer), so splitting M is a net loss.
            mQ = [big.tile([128, 4, 1024], bf16, name=f"m{i}", tag=f"m{i}")
                  for i in range(2)]
            for ac in range(8):
                for bb in range(2):
                    psum = pso.tile([128, 512], f32, name=f"pm{ac}_{bb}",
                                    tag=f"o{bb}")
                    for ec in range(8):
                        nc.tensor.matmul(
                            psum[:], wq_sl(ec, ac * 128, 128),
                            wk_sl(ec, bb * 512, 512),
                            start=(ec == 0), stop=(ec == 7))
                    nc.scalar.copy(
                        mQ[ac // 4][:, ac % 4, bb * 512:(bb + 1) * 512],
                        psum[:])

            def m_sl(ac, b0, n):
                return mQ[ac // 4][:, ac % 4, b0:b0 + n]

            # emb_nat K-tiles 0..15 overwrite wq/wk (dead after M).
            # en[k] holds K-tiles 4k..4k+3 as [128part, 4, 1024d].
            # (en[4..7] alias m/eq and are created after the qt loop so the
            # pool's per-tag alias order matches program order.)
            en_tags = ["wq0", "wq1", "wk0", "wk1", "m0", "m1", "eq0", "eq1"]
            en = [big.tile([128, 4, 1024], bf16, name=f"en{k}",
                           tag=en_tags[k]) for k in range(4)]
            for k in range(4):
                for j in range(4):
                    nc.gpsimd.dma_start(en[k][:, j, :],
                                        ennat_in[4 * k + j, :, :])

            def en_sl(kt, d0, n):      # [128k, n] slice of natural emb
                return en[kt // 4][:, kt % 4, d0:d0 + n]

            # ------------- qt = (emb_q M)^T, kept in SBUF ----------------
            qt_sb = [big.tile([128, 1024], bf16, name=f"qt{i}", tag=f"qt{i}")
                     for i in range(16)]
            for qb in range(4):
                if qb >= 2:
                    eqs.append(big.tile([128, 4, 1024], bf16, name=f"eq{qb}",
                                        tag=f"eq{qb % 2}"))
                    nc.gpsimd.dma_start(eqs[qb][:, :, :],
                                        embq_in[qb, :, :, :])
                eq = eqs[qb]
                half, off = qb // 2, (qb % 2) * 512
                for bc in range(8):
                    psum = pso.tile([128, 512], f32, name=f"pq{qb}_{bc}",
                                    tag=f"o{bc % 2}")
                    for ac in range(8):
                        nc.tensor.matmul(
                            psum[:], m_sl(ac, bc * 128, 128),
                            eq[:, ac // 2, (ac % 2) * 512:(ac % 2) * 512 + 512],
                            start=(ac == 0), stop=(ac == 7))
                    nc.scalar.copy(qt_sb[bc * 2 + half][:, off:off + 512],
                                   psum[:])

            if debug:
                nc.gpsimd.dma_start(dbgq_d[:, :], qt_sb[0][:, 0:128])

            # emb_nat K-tiles 16..31 overwrite m/eq (dead after qt).
            en += [big.tile([128, 4, 1024], bf16, name=f"en{k}",
                            tag=en_tags[k]) for k in range(4, 8)]
            for k in range(4, 8):
                for j in range(4):
                    nc.gpsimd.dma_start(en[k][:, j, :],
                                        ennat_in[4 * k + j, :, :])

            # ---------------- attention ----------------
            # Emission order alternates long and short slots so a short
            # slot's boundary latency hides under the following long slot's
            # score stream.
            order = []
            for i in range(NSLOT // 2):
                order += [NSLOT - 1 - i, i]

            # U^T/l accumulation trails the score stream by two tiles (the
            # pend queue), and each slot's second stage (U Wv^T) is emitted
            # two further drains later so its PSUM->SBUF copies (vector/
            # gpsimd) complete under the next slot's scores.  PSUM is bank-
            # granular: utg 2 banks + o0/o1 2 + s x2 2 + l x2 2 = 8, so
            # the U^T group is single-buffered (per-slice WAR deps let
            # the next slot's accumulation start as each copy completes).
            # The two l accumulators must be separate tiles: column-sliced
            # sharing of one PSUM tile corrupts the neighboring column.
            pend = []
            stage2q = []

            def emit_stage2(j, utsb, r_sb):
                row = j * CHUNK
                for eb in range(2):
                    op = pso.tile([128, 512], f32, name=f"po{j}_{eb}",
                                  tag=f"o{eb}")
                    for dc in range(8):
                        nc.tensor.matmul(
                            op[:], utsb[dc][:],
                            wv_sl(dc, eb * 512, 512),
                            start=(dc == 0), stop=(dc == 7))
                    o_sb = outs.tile([128, 512], bf16, name=f"o{j}_{eb}",
                                     tag=f"os{eb}")
                    if eb == 0:
                        nc.scalar.activation(o_sb[:], op[:], Copy, bias=0.0,
                                             scale=r_sb[:])
                    else:
                        nc.vector.tensor_scalar_mul(o_sb[:], op[:], r_sb[:])
                    nc.sync.dma_start(
                        out_d[row:row + CHUNK, eb * 512:(eb + 1) * 512],
                        o_sb[:])

            def drain_one():
                wt, kt, j, grp, ut, l_ps, nkt = pend.pop(0)
                first, last = kt == 0, kt == nkt - 1
                for ds in range(8):
                    # start=True clears has_written for the WHOLE PSUM bank,
                    # not just this region, so only the first region of each
                    # bank (ds 0 and 4) may carry it.  The bank-wide clear
                    # leaves the other regions' bits unset, so their first
                    # write overwrites (not accumulates) as needed.
                    nc.tensor.matmul(
                        ut[:, ds * 128:(ds + 1) * 128],
                        en_sl(kt, ds * 128, 128), wt[:],
                        start=(first and ds % 4 == 0), stop=last,
                        skip_group_check=True)
                # l rowsum shares wt as the stationary operand; issued last
                # so its weight load prefetches under the U^T streams.
                nc.tensor.matmul(l_ps[:], wt[:], ones[:],
                                 start=first, stop=last)
                if last:
                    r_sb = smalls.tile([128, 1], f32, name=f"r{j}",
                                       tag=f"r{grp}")
                    nc.vector.reciprocal(r_sb[:], l_ps[:])
                    utsb = [big.tile([128, 128], bf16, name=f"uts{j}_{ds}",
                                     tag=f"uts{ds}") for ds in range(8)]
                    for ds in range(8):
                        if ds % 2 == 0:
                            nc.vector.tensor_copy(
                                utsb[ds][:], ut[:, ds * 128:(ds + 1) * 128])
                        else:
                            nc.scalar.copy(
                                utsb[ds][:], ut[:, ds * 128:(ds + 1) * 128])
                    if debug:
                        nc.sync.dma_start(dbgr_d[j, :, :], r_sb[:])
                        nc.gpsimd.dma_start(dbgu_d[j, :, :], utsb[0][:])
                    stage2q.append([2, (j, utsb, r_sb)])
                for e2 in stage2q:
                    e2[0] -= 1
                while stage2q and stage2q[0][0] <= 0:
                    _, (j2, u2, r2) = stage2q.pop(0)
                    emit_stage2(j2, u2, r2)

            for e, j in enumerate(order):
                nkt = NKT[j]
                h, c0 = j // 8, (j % 8) * CHUNK
                qt_tiles = [qt_sb[ec * 2 + h][:, c0:c0 + CHUNK]
                            for ec in range(8)]
                grp = e % 2
                ut = put.tile([128, 1024], f32, name=f"ut{j}", tag="utg")
                l_ps = pl_pool.tile([128, 1], f32, name=f"l{j}",
                                    tag=f"l{grp}")

                for kt in range(nkt):
                    s_ps = ps_pool.tile([128, CHUNK], f32, name=f"s{j}_{kt}",
                                        tag="s")
                    for dc in range(8):
                        nc.tensor.matmul(
                            s_ps[:], embt_sl(dc, kt * 128, 128),
                            qt_tiles[dc], start=(dc == 0), stop=(dc == 7))

                    wt = wts.tile([128, CHUNK], bf16, name=f"w{j}_{kt}",
                                  tag="wts")
                    nc.scalar.activation(wt[:], s_ps[:], Exp, bias=0.0,
                                         scale=INV_SQRT_D)
                    if kt >= nkt - 2:
                        nc.vector.tensor_mul(wt[:], wt[:],
                                             mask_sb[kt - (nkt - 2)][:])
                    if debug and kt == 0:
                        nc.gpsimd.dma_start(dbgw_d[j, :, :], wt[:])
                    pend.append((wt, kt, j, grp, ut, l_ps, nkt))
                    if len(pend) > 2:
                        drain_one()
            while pend:
                drain_one()
            while stage2q:
                _, (j2, u2, r2) = stage2q.pop(0)
                emit_stage2(j2, u2, r2)

    return nc


_CACHED = {}


def _get_graph():
    if "nc" not in _CACHED:
        _install_patches()
        _CACHED["nc"] = _build_graph()
    return _CACHED["nc"]


# ---------------------------------------------------------------------------
# Host-side staging (layout only)
# ---------------------------------------------------------------------------

def _chunks(parity):
    return [2 * j + parity for j in range(NSLOT)]


def _masks(parity):
    # mask tile t (t=0,1) applies to K-tile (2j+t) of slot j:
    # keep iff 128*parity + x >= 128*t + r  (r = k row in partition dim,
    # x = q col in free dim).
    m = np.zeros((2, 128, CHUNK), dtype=np.float32)
    r = np.arange(128)[:, None]
    x = np.arange(CHUNK)[None, :]
    for t in range(2):
        m[t] = ((128 * parity + x) >= (128 * t + r)).astype(np.float32)
    return m.astype(BF16)


def _quads(w):
    # [1024, 1024] row-chunked to [2, 128, 4, 1024] quad layout
    return np.ascontiguousarray(
        w.reshape(2, 4, 128, 1024).transpose(0, 2, 1, 3))


def kernel(embeddings, Wq, Wk, Wv):
    embeddings = np.asarray(embeddings, dtype=np.float32)
    Wq = np.asarray(Wq, dtype=np.float32)
    Wk = np.asarray(Wk, dtype=np.float32)
    Wv = np.asarray(Wv, dtype=np.float32)

    nc = _get_graph()
    from concourse.bass_utils import run_bass_kernel_spmd

    wqh = _quads(Wq).astype(BF16)
    wkh = _quads(Wk).astype(BF16)
    wvh = _quads(np.ascontiguousarray(Wv.T)).astype(BF16)
    masks_by_par = [_masks(0), _masks(1)]

    in_maps = []
    for c in range(NCORES):
        b, par = divmod(c, 2)
        emb_b = embeddings[b]
        embT = emb_b.T                                    # [1024, 4096]
        embTq = np.ascontiguousarray(
            embT.reshape(8, 128, 4, 1024).transpose(2, 1, 0, 3)).astype(BF16)
        ennat = np.ascontiguousarray(
            emb_b.reshape(32, 128, 1024)).astype(BF16)
        rows = np.concatenate(
            [np.arange(g * CHUNK, (g + 1) * CHUNK) for g in _chunks(par)])
        eqT = np.ascontiguousarray(emb_b[rows].T)         # [1024, 2048]
        embqh = np.ascontiguousarray(
            eqT.reshape(4, 2, 128, 4, 512).transpose(3, 2, 0, 1, 4)
            .reshape(4, 128, 4, 1024)).astype(BF16)
        in_maps.append({
            "embTq": embTq,
            "ennat": ennat,
            "embqh": embqh,
            "wqh": wqh,
            "wkh": wkh,
            "wvh": wvh,
            "masks": masks_by_par[par],
        })

    trace = bool(int(os.environ.get("BASS_KERNEL_TRACE", "0")))
    kwargs = {}
    if trace:
        kwargs["trace"] = _install_ntff_hook()

    res = run_bass_kernel_spmd(nc, in_maps, core_ids=list(range(NCORES)),
                               **kwargs)
    _CACHED["last_result"] = res

    out = np.empty((B, S, D), dtype=np.float32)
    for c in range(NCORES):
        b, par = divmod(c, 2)
        core_out = res.results[c]["out"].astype(np.float32)
        for j, g in enumerate(_chunks(par)):
            out[b, g * CHUNK:(g + 1) * CHUNK] = \
                core_out[j * CHUNK:(j + 1) * CHUNK]
    return out
